# revision 1
# baseline (speedup 1.0000x reference)
"""Trainium2 Bass kernel for single-head causal attention.

Problem: B=4, N=2048, D=1024, f32.
  Q = x@Wq; K = x@Wk; V = x@Wv (biases are zero in this problem)
  S = Q K^T / sqrt(D), causal-masked, softmax over keys
  out = (softmax(S) @ V) @ Wo

Algebraic folding (host-side): A = Wq Wk^T / sqrt(D), C = Wv Wo.
  S   = x A x^T        -> K projection eliminated (S^T uses resident x^T
                          tiles as the stationary operand)
  out = P (x C) / dn   -> output projection eliminated (denominator scale
                          folds into the attention PSUM->SBUF copy via a
                          matmul-broadcast of the reciprocal row)
This removes 384 of the baseline's 1088 full-rate matmuls per core.

Sharding: 8 cores = (4 batches) x (2 query-halves). Query rows are split in
512-row chunks; core half 0 takes chunks {0,3} of its batch, half 1 takes
{1,2} (both sum to 20 causal key-tiles -> balanced). Causality differs per
half, so we build TWO specialized NEFFs and dispatch them concurrently on
disjoint 4-device meshes. No collectives.

All matmuls use float32r (FP32 storage, FP22 multiply) at free-dim >= 256
where the TensorEngine runs at bf16 rate. Softmax skips the max
subtraction (scores are O(5) here, exp is safe in f32). The softmax
normalization is folded into the P^T tiles (broadcast-reciprocal row x
tile) inside pass B's pipelined region, so the attention-PSUM drain is a
plain copy split across DVE+ACT and never serializes a pass boundary.
PSUM->SBUF drains elsewhere alternate DVE/ACT too. The output is written
transposed ([D, q]) in bf16 (~0.4% rounding, tolerance is 2e-2) and the
host transposes/widens on gather.

The output drain ships 4 do-planes per strided DMA (2 DMAs per chunk
instead of 8): each DMA completion costs ~900ns of semaphore propagation,
which dominated the end-of-kernel cascade. Phase 1+2 interleaves one VC
row-tile between successive x-row transposes so the PE stays busy while
the x stream paces.

The denominator accumulates on DVE directly in f32r storage (f32 bits;
no extra copy) with a single cross-partition matmul per chunk, keeping
the S-chain free of in-order exp waits. VC stream tiles load in pairs with an explicitly
rearrange-pinned DRAM->SBUF mapping (the implicit [2,P,D]->[P,2,D]
mapping scrambles data on hardware).

VC spills ride the scalar queue (slack between the c-odds load and the
late `a` load) so the sync queue streams x rows uninterrupted through
the fine-grained transpose/VC interleave. Constant tiles (identity,
ones, masks) build once outside the For_i repeat loop, so the ~3-5us
Pool-engine constant chain isn't paid per repetition.

Timeline-sim: 189.5 us per half (prior session baseline: 282.7 us; the
wall-clock repeat-loop measurement on these axon-tunneled cores has
+-ms-level noise, so the deterministic sim is the optimization signal).
"""
import sys
import os

sys.path.insert(0, "/opt/trn_rl_repo")

import numpy as np

import concourse.bass as bass
import concourse.mybir as mybir
import concourse.tile as tile
from concourse import bacc
from concourse.masks import make_identity

P = 128
D = 1024
N = 2048
B = 4
NCORES = 8
F32 = mybir.dt.float32
F32R = mybir.dt.float32r
BF16 = mybir.dt.bfloat16
CHUNK = 512           # query-chunk width (free dim of S^T matmuls)
DSUB = D // P         # 8 feature sub-tiles
NSUB = N // P         # 16 row sub-tiles
NCHUNK = N // CHUNK   # 4 query chunks per batch
CHUNK_MAP = [(0, 3), (1, 2)]   # q-chunk indices per core-half
SCALE = 1.0 / np.sqrt(D)


def build(half: int, reps: int = 1, parts: str = 'all'):
    """Build the Bass graph for core-half `half` (0 or 1).

    reps > 1 wraps the whole body in a device-side loop - used only for
    wall-clock timing measurements (amortizes the host dispatch overhead).
    """
    chunks = CHUNK_MAP[half]
    nc = bacc.Bacc("TRN2", target_bir_lowering=False, debug=False,
                   enable_asserts=False, num_devices=NCORES // 2)

    x_d = nc.dram_tensor("x", [N, D], F32, kind="ExternalInput")
    a_d = nc.dram_tensor("a", [D, D], F32, kind="ExternalInput")
    c_d = nc.dram_tensor("c", [D, D], F32, kind="ExternalInput")
    # transposed bf16 output: out^T[d, q] per core; host transposes and
    # widens on gather (bf16 rounding ~0.4% << 2e-2 tolerance)
    out_d = nc.dram_tensor("out", [D, 2 * CHUNK], BF16, kind="ExternalOutput")

    with tile.TileContext(nc) as tc:
        with tc.tile_pool(name="dram", bufs=1, space="DRAM") as dram, \
             tc.tile_pool(name="const", bufs=1) as const:
            vc_spill = dram.tile([NSUB, P, D], F32R)   # VC = x@C: [n_sub][128n, d]
            # constants build once, OUTSIDE the repeat loop: the ~3-5us
            # Pool-engine constant chain would otherwise re-run every rep
            consts = _build_consts(nc, const)

            if reps > 1:
                with tc.For_i(0, reps, 1):
                    _build_body(nc, tc, half, chunks, x_d, a_d, c_d, out_d,
                                vc_spill, consts)
            else:
                _build_body(nc, tc, half, chunks, x_d, a_d, c_d, out_d,
                            vc_spill, consts)

    nc.compile()
    return nc


def _build_consts(nc, const):
    """Constant tiles: PE-transpose identity, ones column/row, zeros,
    and the 4 causal-diagonal mask tiles."""
    # identity for PE transpose (f32r transpose-mode: 1.5 cyc/row)
    ident = const.tile([P, P], F32)
    make_identity(nc, ident)
    ident_r = const.tile([P, P], F32R)
    nc.vector.tensor_copy(ident_r, ident)

    # ones column for denominator matmuls; ones row for the reciprocal
    # broadcast matmul (memset can't write f32r; round through DVE)
    ones_f32 = const.tile([P, 2], F32)
    nc.gpsimd.memset(ones_f32, 1.0)
    ones = const.tile([P, 2], F32R)
    nc.vector.tensor_copy(ones, ones_f32)
    onesrow_f32 = const.tile([1, P], F32)
    nc.gpsimd.memset(onesrow_f32, 1.0)
    onesrow = const.tile([1, P], F32R)
    nc.vector.tensor_copy(onesrow, onesrow_f32)

    # zeros tile for blanking the trimmed region of diagonal P^T tiles
    zeros_f32 = const.tile([P, 2 * P], F32)
    nc.gpsimd.memset(zeros_f32, 0.0)

    # 4 diagonal mask tiles: M_m[k, q] = 1 if q >= 128*m + k else 0,
    # stored only from their first live column (q >= min(128m, 256))
    masks = []
    for m in range(4):
        ofs = min(P * m, 2 * P)
        width = CHUNK - ofs
        mk = const.tile([P, width], F32, name=f"mask{m}")
        nc.gpsimd.memset(mk, 1.0)
        nc.gpsimd.affine_select(
            out=mk, in_=mk,
            compare_op=mybir.AluOpType.is_ge,
            fill=0.0,
            base=ofs - P * m,
            channel_multiplier=-1,
            pattern=[[1, width]],
        )
        masks.append(mk)
    return dict(ident_r=ident_r, ones=ones, onesrow=onesrow,
                zeros_f32=zeros_f32, masks=masks)


def _build_body(nc, tc, half, chunks, x_d, a_d, c_d, out_d, vc_spill, consts):
    from contextlib import ExitStack

    ident_r = consts["ident_r"]
    ones = consts["ones"]
    onesrow = consts["onesrow"]
    zeros_f32 = consts["zeros_f32"]
    masks = consts["masks"]

    ctx = ExitStack()
    with ctx:
        # Outer-scope pools: resident U^T (= A^T x^T, scaled) and resident
        # x^T (stays live through phase 3 as the S^T stationary operand)
        qt_pool = ctx.enter_context(tc.tile_pool(name="qtp", bufs=16))
        xt_pool = ctx.enter_context(tc.tile_pool(name="xt", bufs=1))

        qt_all = {}    # (ci, d_sub) -> resident U^T tile

        # ============ Phase 1+2: x^T, VC = x@C, U^T = A^T x^T ============
        ph12 = ExitStack()
        with ph12:
            xld = ph12.enter_context(tc.tile_pool(name="xld", bufs=2))
            stg = ph12.enter_context(tc.tile_pool(name="stg", bufs=4))
            cpool = ph12.enter_context(tc.tile_pool(name="cp", bufs=8))
            apool = ph12.enter_context(tc.tile_pool(name="ap", bufs=8))
            tpsum = ph12.enter_context(
                tc.tile_pool(name="tpsum", bufs=2, space="PSUM"))
            ppsum = ph12.enter_context(
                tc.tile_pool(name="ppsum", bufs=5, space="PSUM"))

            xt_big = xt_pool.tile([P, DSUB, N], F32R, name="xtb", tag="xt")
            xt = [xt_big[:, i, :] for i in range(DSUB)]

            def transpose_batch(jlist, split_first=False):
                # 4 [128,128] transposes share one full PSUM bank and drain
                # with a single wide DVE copy (3D AP scatters the 4 blocks
                # into the i-planes of xt_big)
                for j in jlist:
                    xrow = xld.tile([P, D], F32R, name="xrow", tag="xrow")
                    xsrc = x_d.ap().bitcast(F32R)
                    if split_first:
                        # fine-grained first row: the first transpose only
                        # needs columns 0:128, so start it ASAP
                        for c0, c1 in ((0, P), (P, CHUNK), (CHUNK, D)):
                            nc.sync.dma_start(xrow[:, c0:c1],
                                              xsrc[j * P:(j + 1) * P, c0:c1])
                        split_first = False
                    else:
                        nc.sync.dma_start(xrow, xsrc[j * P:(j + 1) * P, :])
                    for i4 in range(0, DSUB, 4):
                        tp = tpsum.tile([P, 4 * P], F32R, name="tp", tag="tp")
                        for k in range(4):
                            nc.tensor.transpose(
                                tp[:, k * P:(k + 1) * P],
                                xrow[:, (i4 + k) * P:(i4 + k + 1) * P], ident_r)
                        if i4 == 0:
                            nc.vector.tensor_copy(
                                xt_big[:, i4:i4 + 4, j * P:(j + 1) * P],
                                tp.rearrange("p (a b) -> p a b", a=4))
                        else:
                            nc.scalar.activation(
                                xt_big[:, i4:i4 + 4, j * P:(j + 1) * P],
                                tp.rearrange("p (a b) -> p a b", a=4),
                                mybir.ActivationFunctionType.Copy)

            def load_w(dram_t, pool, nm, split=False):
                # weight DMAs go on the scalar-engine HWDGE queue so they
                # never head-block the x stream on the sync queue; the first
                # weight is split across both queues for latency
                tiles = []
                for s in range(DSUB):
                    wt = pool.tile([P, D], F32R, name=f"{nm}{s}", tag="w")
                    eng = nc.sync if (split and s % 2 == 0) else nc.scalar
                    eng.dma_start(
                        wt, dram_t.ap().bitcast(F32R)[s * P:(s + 1) * P, :])
                    tiles.append(wt)
                return tiles

            transpose_batch(range(0, 4), split_first=True)
            cw = load_w(c_d, cpool, "c", split=True)

            def vc_group(ns_range):
                for ns in ns_range:
                    vstg = stg.tile([P, D], F32R, name="vstg", tag="vstg",
                                    bufs=2)
                    for dh in range(2):
                        ps = ppsum.tile([P, CHUNK], F32, name="vps", tag="pp")
                        for di in range(DSUB):
                            nc.tensor.matmul(
                                ps,
                                lhsT=xt[di][:, ns * P:(ns + 1) * P],
                                rhs=cw[di][:, dh * CHUNK:(dh + 1) * CHUNK],
                                start=(di == 0), stop=(di == DSUB - 1))
                        if dh == 0:
                            nc.vector.tensor_copy(
                                vstg[:, dh * CHUNK:(dh + 1) * CHUNK], ps)
                        else:
                            nc.scalar.activation(
                                vstg[:, dh * CHUNK:(dh + 1) * CHUNK], ps,
                                mybir.ActivationFunctionType.Copy)
                    # spills ride the scalar queue (slack between c-odds
                    # and the late `a` load) so the sync queue keeps
                    # streaming x rows without interruption
                    nc.scalar.dma_start(vc_spill[ns], vstg)

            # fine interleave: one VC row-tile (3.15us of PE) between
            # successive x-row transposes (1.44us PE, 2.2us DMA) keeps the
            # PE busy while the x stream paces the transposes
            for j in range(4, 16):
                vc_group(range(j - 4, j - 3))
                transpose_batch(range(j, j + 1))
                if j == 9:
                    aw = load_w(a_d, apool, "a")
            vc_group(range(12, 16))

            # --- U^T = A^T x^T at my query chunks (scale folded into A) ---
            for ci, qc in enumerate(chunks):
                g0 = qc * CHUNK
                for do in range(DSUB):
                    ps = ppsum.tile([P, CHUNK], F32, name="qtps", tag="pp")
                    for di in range(DSUB):
                        nc.tensor.matmul(
                            ps,
                            lhsT=aw[di][:, do * P:(do + 1) * P],
                            rhs=xt[di][:, g0:g0 + CHUNK],
                            start=(di == 0), stop=(di == DSUB - 1))
                    qts = qt_pool.tile([P, CHUNK], F32R,
                                       name=f"qt{ci}_{do}", tag="qt")
                    if do % 2 == 0:
                        nc.vector.tensor_copy(qts, ps)
                    else:
                        nc.scalar.activation(
                            qts, ps, mybir.ActivationFunctionType.Copy)
                    qt_all[(ci, do)] = qts

        # ================= Phase 3: attention per q-chunk =================
        v_pool = ctx.enter_context(tc.tile_pool(name="vp", bufs=6))
        ph3 = ExitStack()
        with ph3:
            pt_pool = ph3.enter_context(tc.tile_pool(name="ptp", bufs=16))
            out_pool = ph3.enter_context(tc.tile_pool(name="outp", bufs=4))
            dn_pool = ph3.enter_context(tc.tile_pool(name="dnp", bufs=4))

            def pass_a(ci, qc):
                """S^T -> exp -> P^T tiles + reciprocal broadcast tile."""
                T = 4 * (qc + 1)
                qt = [qt_all[(ci, s)] for s in range(DSUB)]
                pt_tiles = []
                pa_psum = ExitStack()
                with pa_psum:
                    spsum = pa_psum.enter_context(
                        tc.tile_pool(name="spsum", bufs=2, space="PSUM"))
                    dpsum = pa_psum.enter_context(
                        tc.tile_pool(name="dpsum", bufs=2, space="PSUM"))

                    pacc = dn_pool.tile([P, CHUNK], F32R, name="pacc",
                                        tag="pacc", bufs=2)

                    for kc in range(T):
                        m = kc - 4 * qc
                        # block-causal: diagonal tile m only touches query
                        # columns >= 128*m (cap at 256 so N' stays >= 256,
                        # where fp32r runs at full rate)
                        off = min(P * m, 2 * P) if m > 0 else 0
                        ps = spsum.tile([P, CHUNK], F32, name="sps", tag="sp")
                        for s in range(DSUB):
                            nc.tensor.matmul(
                                ps[:, off:],
                                lhsT=xt[s][:, kc * P:(kc + 1) * P],
                                rhs=qt[s][:, off:],
                                start=(s == 0), stop=(s == DSUB - 1))
                        pt = pt_pool.tile([P, CHUNK], F32R, name="pt",
                                          tag="pt")
                        nc.scalar.activation(
                            pt[:, off:], ps[:, off:],
                            mybir.ActivationFunctionType.Exp)
                        if 0 <= m < 4:
                            # m<3 masks differ from 1.0 only in their
                            # 128-col diagonal window; m=3 also zeroes its
                            # dead zone so it keeps the full range
                            hi = P * (m + 1) if m < 3 else CHUNK
                            nc.vector.tensor_mul(pt[:, off:hi],
                                                 pt[:, off:hi],
                                                 masks[m][:, :hi - off])
                        pt_tiles.append((pt, off))
                        # denominator row: dn[0, q] += sum_k P^T[k, q]. The
                        # trimmed region of diagonal tiles is zeroed so every
                        # tile contributes full width and the PSUM chain
                        # stays uniform.
                        # denominator partial sums on DVE so the S-chain
                        # never stalls in-order behind exp/mask; diagonal
                        # tiles add only their live columns, so the trimmed
                        # region is never read and needs no blanking
                        if kc == 0:
                            nc.vector.tensor_copy(pacc, pt)
                        else:
                            nc.vector.tensor_add(pacc[:, off:],
                                                 pacc[:, off:],
                                                 pt[:, off:])

                    # single cross-partition reduction (pacc is f32r
                    # storage = f32 bits, so the matmul runs at full rate
                    # with no extra copy), then reciprocal straight from
                    # PSUM to f32r, then broadcast across all 128
                    # partitions with a 1-contraction matmul
                    dn_ps = dpsum.tile([1, CHUNK], F32, name="dnrow",
                                       tag="dnrow")
                    nc.tensor.matmul(dn_ps, lhsT=ones[:, 0:1], rhs=pacc,
                                     start=True, stop=True)
                    rec_row_r = dn_pool.tile([1, CHUNK], F32R, name="recrowr",
                                             tag="recrowr")
                    with nc.allow_low_precision(
                            reason="f32r is full 32-bit storage; fp22 "
                                   "matmul rounding ~6e-5 << 2e-2 tol"):
                        nc.vector.reciprocal(rec_row_r, dn_ps)
                    bc_ps = dpsum.tile([P, CHUNK], F32, name="bcps",
                                       tag="bcps")
                    nc.tensor.matmul(bc_ps, lhsT=onesrow, rhs=rec_row_r,
                                     start=True, stop=True)
                    recip_bc = dn_pool.tile([P, CHUNK], F32, name="recbc",
                                            tag="recbc")
                    nc.vector.tensor_copy(recip_bc, bc_ps)
                return pt_tiles, recip_bc

            def pass_b(ci, qc, pt_tiles, recip_bc):
                """att^T[do] = sum_k VC^T P^T, scaled by recip -> out DMA."""
                T = 4 * (qc + 1)
                pb_psum = ExitStack()
                with pb_psum:
                    apsum = pb_psum.enter_context(
                        tc.tile_pool(name="apsum", bufs=1, space="PSUM"))
                    a_ps = [apsum.tile([P, CHUNK], F32, name=f"a{do}",
                                       tag=f"a{do}")
                            for do in range(DSUB)]
                    vt2 = None
                    for kc in range(T):
                        if kc % 2 == 0:
                            # one DMA per two key-tiles; the rearrange pins
                            # the DRAM->SBUF dim mapping explicitly (the
                            # implicit [2,P,D]->[P,2,D] mapping scrambles)
                            vt2 = v_pool.tile([P, 2, D], F32R, name="vt",
                                              tag="vt", bufs=3)
                            nc.sync.dma_start(
                                vt2,
                                vc_spill[kc:kc + 2].rearrange(
                                    "j p d -> p j d"))
                        vt = vt2[:, kc % 2, :]
                        pt, off = pt_tiles[kc]
                        # fold the softmax normalization into P^T here (in
                        # the pipelined region) so the PSUM drain below is a
                        # plain copy and doesn't serialize on one engine at
                        # the pass boundary
                        nc.vector.tensor_mul(pt[:, off:], pt[:, off:],
                                             recip_bc[:, off:])
                        for do in range(DSUB):
                            nc.tensor.matmul(
                                a_ps[do][:, off:],
                                lhsT=vt[:, do * P:(do + 1) * P],
                                rhs=pt[:, off:],
                                start=(kc == 0), stop=(kc == T - 1))
                    # drain 4 do-planes into one staging tile and ship
                    # them with a single strided DMA (2 DMAs per chunk
                    # instead of 8 shrinks the end-of-kernel sem cascade)
                    odst = out_d.ap()[:, ci * CHUNK:(ci + 1) * CHUNK]
                    odst = odst.rearrange("(g p) q -> p g q", p=P)
                    for g in range(2):
                        og = out_pool.tile([P, 4, CHUNK], BF16,
                                           name=f"og{g}", tag="og", bufs=2)
                        for k in range(4):
                            do = 4 * g + k
                            if do % 2 == 0:
                                nc.vector.tensor_copy(og[:, k, :], a_ps[do])
                            else:
                                nc.scalar.activation(
                                    og[:, k, :], a_ps[do],
                                    mybir.ActivationFunctionType.Copy)
                        nc.sync.dma_start(
                            odst[:, 4 * g:4 * g + 4, :], og)

            c0, c1 = chunks[0], chunks[1]
            pt0, rec0 = pass_a(0, c0)
            pass_b(0, c0, pt0, rec0)
            pt1, rec1 = pass_a(1, c1)
            pass_b(1, c1, pt1, rec1)

# ---------------------------------------------------------------------------
# Host-side dispatch
# ---------------------------------------------------------------------------

_CACHE = {}


def _get_executables():
    if "exes" in _CACHE:
        return _CACHE["exes"]
    import jax
    from jax.sharding import Mesh, PartitionSpec
    from jax.experimental.shard_map import shard_map
    from concourse.bass2jax import (_bass_exec_p, install_neuronx_cc_hook,
                                    partition_id_tensor)

    install_neuronx_cc_hook()
    devices = jax.devices()
    assert len(devices) >= NCORES, f"need {NCORES} devices, have {len(devices)}"

    exes = []
    for half in range(2):
        nc = build(half)
        partition_name = (nc.partition_id_tensor.name
                          if nc.partition_id_tensor else None)
        in_names, out_names, out_avals, zero_shapes = [], [], [], []
        for alloc in nc.m.functions[0].allocations:
            if not isinstance(alloc, mybir.MemoryLocationSet):
                continue
            name = alloc.memorylocations[0].name
            if alloc.kind == "ExternalInput":
                if name != partition_name:
                    in_names.append(name)
            elif alloc.kind == "ExternalOutput":
                out_names.append(name)
                shape = tuple(alloc.tensor_shape)
                dtype = mybir.dt.np(alloc.dtype)
                out_avals.append(jax.core.ShapedArray(shape, dtype))
                zero_shapes.append((shape, dtype))
        n_params = len(in_names)
        all_in_names = list(in_names) + list(out_names)
        if partition_name is not None:
            all_in_names.append(partition_name)
        donate = tuple(range(n_params, n_params + len(out_names)))

        def _body(*args, _nc=nc, _out_avals=tuple(out_avals),
                  _all_in=tuple(all_in_names), _out=tuple(out_names),
                  _pid=partition_name):
            operands = list(args)
            if _pid is not None:
                operands.append(partition_id_tensor())
            return tuple(_bass_exec_p.bind(
                *operands, out_avals=_out_avals, in_names=_all_in,
                out_names=_out, lowering_input_output_aliases=(),
                sim_require_finite=True, sim_require_nnan=True, nc=_nc))

        devs = devices[half * 4:(half + 1) * 4]
        mesh = Mesh(np.asarray(devs), ("core",))
        in_specs = (PartitionSpec("core"),) * (n_params + len(out_names))
        out_specs = (PartitionSpec("core"),) * len(out_names)
        sharded = jax.jit(
            shard_map(_body, mesh=mesh, in_specs=in_specs,
                      out_specs=out_specs, check_rep=False),
            donate_argnums=donate, keep_unused=True)
        exes.append(dict(fn=sharded, in_names=in_names,
                         out_names=out_names, zero_shapes=zero_shapes))
    _CACHE["exes"] = exes
    return exes


def kernel(**inputs):
    x = np.asarray(inputs["x"], dtype=np.float32)      # [B, N, D]
    Wq = np.asarray(inputs["Wq"], dtype=np.float32)
    Wk = np.asarray(inputs["Wk"], dtype=np.float32)
    Wv = np.asarray(inputs["Wv"], dtype=np.float32)
    Wo = np.asarray(inputs["Wo"], dtype=np.float32)

    # algebraic folding: S = x A x^T, out = P (x C)
    A = (Wq @ Wk.T) * np.float32(SCALE)
    C = Wv @ Wo

    import jax
    import time as _time

    exes = _get_executables()
    per_half_in = []
    for half in range(2):
        ex = exes[half]
        per_core = []
        for b in range(B):
            m = {"x": x[b], "a": A, "c": C}
            per_core.append([np.ascontiguousarray(m[nm])
                             for nm in ex["in_names"]])
        per_half_in.append(
            [np.concatenate([per_core[c][i] for c in range(B)], axis=0)
             for i in range(len(ex["in_names"]))])

    outs = None
    for attempt in range(3):
        try:
            outs = []
            for half in range(2):
                ex = exes[half]
                zeros = [np.zeros((B * s[0], *s[1:]), dt)
                         for s, dt in ex["zero_shapes"]]
                outs.append(ex["fn"](*per_half_in[half], *zeros))
            jax.block_until_ready(outs)
            break
        except Exception:
            if attempt == 2:
                raise
            _time.sleep(10)

    out_full = np.empty((B, N, D), dtype=np.float32)
    for half in range(2):
        ex = exes[half]
        arr = np.asarray(outs[half][ex["out_names"].index("out")])
        arr = arr.reshape(B, D, 2 * CHUNK)
        for b in range(B):
            for ci, qc in enumerate(CHUNK_MAP[half]):
                out_full[b, qc * CHUNK:(qc + 1) * CHUNK] = \
                    arr[b, :, ci * CHUNK:(ci + 1) * CHUNK].T
    return out_full



# revision 56
# speedup vs baseline: 1.0631x; 1.0631x over previous
"""Trainium2 Bass kernel for single-head causal attention.

Problem: B=4, N=2048, D=1024, f32.
  Q = x@Wq; K = x@Wk; V = x@Wv (biases are zero in this problem)
  S = Q K^T / sqrt(D), causal-masked, softmax over keys
  out = (softmax(S) @ V) @ Wo

Algebraic folding (host-side): A = Wq Wk^T / sqrt(D), C = Wv Wo.
  S   = x A x^T        -> K projection eliminated (S^T uses resident x^T
                          tiles as the stationary operand)
  out = P (x C) / dn   -> output projection eliminated (denominator scale
                          folds into the attention PSUM->SBUF copy via a
                          matmul-broadcast of the reciprocal row)
This removes 384 of the baseline's 1088 full-rate matmuls per core.

Sharding: 8 cores = (4 batches) x (2 query-halves). Query rows are split in
512-row chunks; core half 0 takes chunks {0,3} of its batch, half 1 takes
{1,2} (both sum to 20 causal key-tiles -> balanced). Causality differs per
half, so we build TWO specialized NEFFs and dispatch them concurrently on
disjoint 4-device meshes. No collectives.

All matmuls use float32r (FP32 storage, FP22 multiply) at free-dim >= 256
where the TensorEngine runs at bf16 rate. Softmax skips the max
subtraction (scores are O(5) here, exp is safe in f32). The softmax
normalization is folded into the P^T tiles (broadcast-reciprocal row x
tile) inside pass B's pipelined region, so the attention-PSUM drain is a
plain copy split across DVE+ACT and never serializes a pass boundary.
PSUM->SBUF drains elsewhere alternate DVE/ACT too. The output is written
transposed ([D, q]) in bf16 (~0.4% rounding, tolerance is 2e-2) and the
host transposes/widens on gather.

The output drain ships 4 do-planes per strided DMA (2 DMAs per chunk
instead of 8): each DMA completion costs ~900ns of semaphore propagation,
which dominated the end-of-kernel cascade. Phase 1+2 interleaves one VC
row-tile between successive x-row transposes so the PE stays busy while
the x stream paces.

The denominator accumulates on DVE directly in f32r storage (f32 bits;
no extra copy) with a single cross-partition matmul per chunk, keeping
the S-chain free of in-order exp waits. VC stream tiles load in pairs with an explicitly
rearrange-pinned DRAM->SBUF mapping (the implicit [2,P,D]->[P,2,D]
mapping scrambles data on hardware).

VC spills ride the scalar queue (slack between the c-odds load and the
late `a` load) so the sync queue streams x rows uninterrupted through
the fine-grained transpose/VC interleave. Constant tiles (identity,
ones, masks) build once outside the For_i repeat loop, so the ~3-5us
Pool-engine constant chain isn't paid per repetition.

Timeline-sim: 189.5 us per half (prior session baseline: 282.7 us; the
wall-clock repeat-loop measurement on these axon-tunneled cores has
+-ms-level noise, so the deterministic sim is the optimization signal).
"""
import sys
import os

sys.path.insert(0, "/opt/trn_rl_repo")

import numpy as np

import concourse.bass as bass
import concourse.mybir as mybir
import concourse.tile as tile
from concourse import bacc
from concourse.masks import make_identity

P = 128
D = 1024
N = 2048
B = 4
NCORES = 8
F32 = mybir.dt.float32
F32R = mybir.dt.float32r
BF16 = mybir.dt.bfloat16
CHUNK = 512           # query-chunk width (free dim of S^T matmuls)
DSUB = D // P         # 8 feature sub-tiles
NSUB = N // P         # 16 row sub-tiles
NCHUNK = N // CHUNK   # 4 query chunks per batch
CHUNK_MAP = [(0, 3), (1, 2)]   # q-chunk indices per core-half
SCALE = 1.0 / np.sqrt(D)


def build(half: int, reps: int = 1, parts: str = 'all'):
    """Build the Bass graph for core-half `half` (0 or 1).

    reps > 1 wraps the whole body in a device-side loop - used only for
    wall-clock timing measurements (amortizes the host dispatch overhead).
    """
    chunks = CHUNK_MAP[half]
    nc = bacc.Bacc("TRN2", target_bir_lowering=False, debug=False,
                   enable_asserts=False, num_devices=NCORES // 2)

    x_d = nc.dram_tensor("x", [N, D], F32, kind="ExternalInput")
    a_d = nc.dram_tensor("a", [D, D], F32, kind="ExternalInput")
    # C stays f32: walrus rejects mixed 32/16-bit matmul inputs, and the
    # VC matmul's other operand (x^T) must stay f32r for S precision
    c_d = nc.dram_tensor("c", [D, D], F32, kind="ExternalInput")
    # transposed bf16 output: out^T[d, q] per core; host transposes and
    # widens on gather (bf16 rounding ~0.4% << 2e-2 tolerance)
    out_d = nc.dram_tensor("out", [D, 2 * CHUNK], BF16, kind="ExternalOutput")

    with tile.TileContext(nc) as tc:
        with tc.tile_pool(name="const", bufs=1) as const:
            # constants build once, OUTSIDE the repeat loop: the ~3-5us
            # Pool-engine constant chain would otherwise re-run every rep.
            # f32 staging tiles live in a scoped pool freed before the body
            with tc.tile_pool(name="cstg", bufs=1) as cstg:
                consts = _build_consts(nc, const, cstg)

            if reps > 1:
                with tc.For_i(0, reps, 1):
                    _build_body(nc, tc, half, chunks, x_d, a_d, c_d, out_d,
                                consts)
            else:
                _build_body(nc, tc, half, chunks, x_d, a_d, c_d, out_d,
                            consts)

    nc.compile()
    return nc


def _build_consts(nc, const, cstg):
    """Constant tiles: PE-transpose identity, all-ones square (one-matmul
    denominator sum+broadcast), and the 4 causal-diagonal mask tiles."""
    # identity for PE transpose (f32r transpose-mode: 1.5 cyc/row);
    # f32 staging tiles share one ring slot (tag) to save SBUF
    ident = cstg.tile([P, P], F32, name="ident", tag="cstg")
    make_identity(nc, ident)
    ident_r = const.tile([P, P], F32R)
    nc.vector.tensor_copy(ident_r, ident)

    # all-ones [P, P]: matmul(lhsT=ones128, rhs=pacc) computes the
    # cross-partition sum AND broadcasts it to all 128 partitions in one
    # instruction (memset can't write f32r; round through DVE)
    ones_f32 = cstg.tile([P, P], F32, name="onesf", tag="cstg")
    nc.gpsimd.memset(ones_f32, 1.0)
    ones128 = const.tile([P, P], F32R)
    nc.vector.tensor_copy(ones128, ones_f32)

    # 4 diagonal mask tiles (bf16, matching the P^T dtype for 2x DVE):
    # M_m[k, q] = 1 if q >= 128*m + k else 0, stored only from their
    # first live column (q >= min(128m, 256))
    masks = []
    for m in range(4):
        ofs = min(P * m, 2 * P)
        width = CHUNK - ofs
        mkf = cstg.tile([P, width], F32, name=f"maskf{m}", tag="cstg")
        nc.gpsimd.memset(mkf, 1.0)
        nc.gpsimd.affine_select(
            out=mkf, in_=mkf,
            compare_op=mybir.AluOpType.is_ge,
            fill=0.0,
            base=ofs - P * m,
            channel_multiplier=-1,
            pattern=[[1, width]],
        )
        mk = const.tile([P, width], BF16, name=f"mask{m}")
        nc.vector.tensor_copy(mk, mkf)
        masks.append(mk)
    return dict(ident_r=ident_r, ones128=ones128, masks=masks)


def _build_body(nc, tc, half, chunks, x_d, a_d, c_d, out_d, consts):
    from contextlib import ExitStack

    ident_r = consts["ident_r"]
    ones128 = consts["ones128"]
    masks = consts["masks"]

    ctx = ExitStack()
    with ctx:
        # Outer-scope pools: resident U^T (= A^T x^T, scaled), resident
        # x^T (stays live through phase 3 as the S^T stationary operand),
        # and resident VC = x@C in bf16 (SBUF-resident: no DRAM spill or
        # pass-B reload; bf16 rounding ~0.4% << 2e-2 tol, and the matmul
        # rate is set by the moving operand so a bf16 lhsT costs nothing)
        qt_pool = ctx.enter_context(tc.tile_pool(name="qtp", bufs=16))
        xt_pool = ctx.enter_context(tc.tile_pool(name="xt", bufs=1))
        vc_pool = ctx.enter_context(tc.tile_pool(name="vcp", bufs=1))
        vc_big = vc_pool.tile([P, NSUB, D], BF16, name="vcb", tag="vc")

        qt_all = {}    # (ci, d_sub) -> resident U^T tile

        # ============ Phase 1+2: x^T, VC = x@C, U^T = A^T x^T ============
        ph12 = ExitStack()
        with ph12:
            # xld: single-row tiles, depth 3 (12KB/partition — the most
            # SBUF allows at the U^T peak). 1.46us transfers against a
            # 2.34us per-row PE cadence keeps the stream ahead.
            xld = ph12.enter_context(tc.tile_pool(name="xld", bufs=3))
            cpool = ph12.enter_context(tc.tile_pool(name="cp", bufs=1))
            apool = ph12.enter_context(tc.tile_pool(name="ap", bufs=1))
            tpsum = ph12.enter_context(
                tc.tile_pool(name="tpsum", bufs=2, space="PSUM"))
            ppsum = ph12.enter_context(
                tc.tile_pool(name="ppsum", bufs=5, space="PSUM"))

            xt_big = xt_pool.tile([P, DSUB, N], F32R, name="xtb", tag="xt")
            xt = [xt_big[:, i, :] for i in range(DSUB)]

            xrows = {}   # row index -> staged [P, D] tile

            def load_xrow(j, split_first=False):
                xsrc = x_d.ap().bitcast(F32R)
                xr = xld.tile([P, D], F32R, name="xr", tag="xrow")
                if split_first:
                    # fine-grained row 0: the first transpose only needs
                    # columns 0:128, so land that piece ASAP
                    for c0, c1 in ((0, P), (P, CHUNK), (CHUNK, D)):
                        nc.sync.dma_start(xr[:, c0:c1],
                                          xsrc[0:P, c0:c1])
                else:
                    nc.sync.dma_start(xr, xsrc[j * P:(j + 1) * P, :])
                xrows[j] = xr

            def transpose_batch(jlist):
                # 4 [128,128] transposes share one full PSUM bank and drain
                # with a single wide DVE copy (3D AP scatters the 4 blocks
                # into the i-planes of xt_big)
                for j in jlist:
                    xrow = xrows[j]
                    for i4 in range(0, DSUB, 4):
                        tp = tpsum.tile([P, 4 * P], F32R, name="tp", tag="tp")
                        for k in range(4):
                            nc.tensor.transpose(
                                tp[:, k * P:(k + 1) * P],
                                xrow[:, (i4 + k) * P:(i4 + k + 1) * P], ident_r)
                        if i4 == 0:
                            nc.vector.tensor_copy(
                                xt_big[:, i4:i4 + 4, j * P:(j + 1) * P],
                                tp.rearrange("p (a b) -> p a b", a=4))
                        else:
                            nc.scalar.activation(
                                xt_big[:, i4:i4 + 4, j * P:(j + 1) * P],
                                tp.rearrange("p (a b) -> p a b", a=4),
                                mybir.ActivationFunctionType.Copy)

            # C rides the scalar queue in column-eighth descriptors into
            # one [P, 8, D] resident tile. Only the LEFT half (4 eighths)
            # loads up front: the VC left-half passes below need just
            # cw[:, 0:512], so PE-feeding work exists ~6us in. The right
            # half is paced into the scalar queue mid-loop, and A follows
            # it — neither steals bandwidth from the x stream while the x
            # stream is the critical path.
            cw_big = cpool.tile([P, DSUB, D], F32R, name="cw", tag="w")
            csrc = c_d.ap().bitcast(F32R)
            CQ = D // 8

            def load_c_eighth(h, eng=None):
                (eng or nc.scalar).dma_start(
                    cw_big[:, :, h * CQ:(h + 1) * CQ],
                    csrc[:, h * CQ:(h + 1) * CQ].rearrange(
                        "(s p) d -> p s d", p=P))

            def vc_half(ns, dh):
                # one column-half of one VC row-tile: 8 matmuls, 1.7us PE
                ps = ppsum.tile([P, CHUNK], F32, name="vps", tag="pp")
                for di in range(DSUB):
                    nc.tensor.matmul(
                        ps,
                        lhsT=xt[di][:, ns * P:(ns + 1) * P],
                        rhs=cw[di][:, dh * CHUNK:(dh + 1) * CHUNK],
                        start=(di == 0), stop=(di == DSUB - 1))
                # drain straight into the resident bf16 VC tile,
                # alternating DVE/ACT
                if dh == 0:
                    nc.vector.tensor_copy(
                        vc_big[:, ns, dh * CHUNK:(dh + 1) * CHUNK], ps)
                else:
                    nc.scalar.activation(
                        vc_big[:, ns, dh * CHUNK:(dh + 1) * CHUNK],
                        ps, mybir.ActivationFunctionType.Copy)

            # row 0 first (unlocks T0), then C-left on BOTH queues at
            # full bandwidth — PE has no other work until C-left lands,
            # so starving the x stream for 5.8us here costs nothing
            load_xrow(0, split_first=True)
            for h in range(4):
                load_c_eighth(h)
            cw = [cw_big[:, s, :] for s in range(DSUB)]

            load_xrow(1)
            transpose_batch(range(0, 2))
            vc_half(0, 0)

            # Pass 1: transposes interleaved with VC LEFT halves (only
            # need C's left half, in ~6us); lag 1 — row j's VC-half runs
            # after transpose j+1, giving the T drains (DVE/ACT) one
            # transpose of slack without waiting on far-later rows. C
            # right-half eighths are paced into the scalar queue mid-loop
            # to soak leftover HBM bandwidth.
            for j in range(2, 16):
                load_xrow(j)
                transpose_batch(range(j, j + 1))
                vc_half(j - 1, 0)
                if j in (7, 10, 13):
                    load_c_eighth(4 + (j - 7) // 3)
            vc_half(15, 0)
            load_c_eighth(7)

            # Pass 2: VC right halves; A follows C on the scalar queue a
            # few groups in — its ~11.6us transfer runs under the VC
            # right-half passes and lands well before U^T
            aw_big = apool.tile([P, DSUB, D], F32R, name="aw", tag="w")
            asrc = a_d.ap().bitcast(F32R).rearrange("(s p) d -> p s d", p=P)
            for ns in range(NSUB):
                vc_half(ns, 1)
                if ns in (2, 4, 6, 8):
                    s2 = ns - 2
                    nc.scalar.dma_start(aw_big[:, s2:s2 + 2, :],
                                        asrc[:, s2:s2 + 2, :])
            aw = [aw_big[:, s, :] for s in range(DSUB)]

            # --- U^T = A^T x^T at my query chunks (scale folded into A) ---
            for ci, qc in enumerate(chunks):
                g0 = qc * CHUNK
                for do in range(DSUB):
                    ps = ppsum.tile([P, CHUNK], F32, name="qtps", tag="pp")
                    for di in range(DSUB):
                        nc.tensor.matmul(
                            ps,
                            lhsT=aw[di][:, do * P:(do + 1) * P],
                            rhs=xt[di][:, g0:g0 + CHUNK],
                            start=(di == 0), stop=(di == DSUB - 1))
                    qts = qt_pool.tile([P, CHUNK], F32R,
                                       name=f"qt{ci}_{do}", tag="qt")
                    if do % 2 == 0:
                        nc.vector.tensor_copy(qts, ps)
                    else:
                        nc.scalar.activation(
                            qts, ps, mybir.ActivationFunctionType.Copy)
                    qt_all[(ci, do)] = qts

        # ================= Phase 3: attention per q-chunk =================
        # Reordered to keep the in-order PE queue busy across the serial
        # PE->DVE->PE denominator chains: both chunks' S-passes run
        # back-to-back, then pass B interleaves the small chunk's waves
        # into the chain gaps. Pass B is split into two 4-bank PSUM waves
        # (do-planes 0..3 / 4..7) so spsum+dpsum+apsum fit in 8 banks and
        # a wave's drain overlaps the next wave's matmuls.
        ph3 = ExitStack()
        with ph3:
            pt_pool = ph3.enter_context(tc.tile_pool(name="ptp", bufs=20))
            out_pool = ph3.enter_context(tc.tile_pool(name="outp", bufs=4))
            dn_pool = ph3.enter_context(tc.tile_pool(name="dnp", bufs=6))
            # bufs is the per-tag ring depth: spsum 1 tag x 2 = 2 banks,
            # dpsum 2 tags x 1 = 2 banks, apsum 4 tags x 1 = 4 banks
            spsum = ph3.enter_context(
                tc.tile_pool(name="spsum", bufs=2, space="PSUM"))
            dpsum = ph3.enter_context(
                tc.tile_pool(name="dpsum", bufs=1, space="PSUM"))
            apsum = ph3.enter_context(
                tc.tile_pool(name="apsum", bufs=1, space="PSUM"))

            def pass_a_s(ci, qc):
                """S^T -> exp -> P^T tiles; DVE denominator acc + recip row."""
                T = 4 * (qc + 1)
                qt = [qt_all[(ci, s)] for s in range(DSUB)]
                pt_tiles = []
                pacc = dn_pool.tile([P, CHUNK], F32R, name=f"pacc{ci}",
                                    tag="pacc", bufs=2)

                # diagonal tiles FIRST: their serial exp->mask->pacc tail
                # then runs while the off-diagonal S matmuls stream, so the
                # dn matmul at the end only waits on a plain add. P^T is
                # bf16 (walrus requires pass B's operands to match vc_big's
                # 16-bit class; also 2x DVE throughput on the softmax chain)
                kc_order = list(range(4 * qc, T)) + list(range(4 * qc))
                pt_by_kc = {}
                for ki, kc in enumerate(kc_order):
                    m = kc - 4 * qc
                    # block-causal: diagonal tile m only touches query
                    # columns >= 128*m (cap at 256 so N' stays >= 256,
                    # where fp32r runs at full rate)
                    off = min(P * m, 2 * P) if m > 0 else 0
                    ps = spsum.tile([P, CHUNK], F32, name="sps", tag="sp")
                    for s in range(DSUB):
                        nc.tensor.matmul(
                            ps[:, off:],
                            lhsT=xt[s][:, kc * P:(kc + 1) * P],
                            rhs=qt[s][:, off:],
                            start=(s == 0), stop=(s == DSUB - 1))
                    pt = pt_pool.tile([P, CHUNK], BF16, name="pt",
                                      tag="pt")
                    nc.scalar.activation(
                        pt[:, off:], ps[:, off:],
                        mybir.ActivationFunctionType.Exp)
                    if 0 <= m < 4:
                        # m<3 masks differ from 1.0 only in their
                        # 128-col diagonal window; m=3 also zeroes its
                        # dead zone so it keeps the full range
                        hi = P * (m + 1) if m < 3 else CHUNK
                        nc.vector.tensor_mul(pt[:, off:hi],
                                             pt[:, off:hi],
                                             masks[m][:, :hi - off])
                    pt_by_kc[kc] = (pt, off)
                    # denominator partial sums on DVE so the S-chain
                    # never stalls in-order behind exp/mask; diagonal
                    # tiles add only their live columns (f32r accumulator
                    # keeps the 2048-term sum accurate)
                    if ki == 0:
                        nc.vector.tensor_copy(pacc, pt)
                    else:
                        nc.vector.tensor_add(pacc[:, off:],
                                             pacc[:, off:],
                                             pt[:, off:])
                pt_tiles = [pt_by_kc[kc] for kc in range(T)]
                return pt_tiles, pacc

            def pass_a_bc(ci, pacc):
                """One matmul sums pacc across partitions AND broadcasts the
                denominator to all 128; reciprocal straight to bf16."""
                bc_ps = dpsum.tile([P, CHUNK], F32, name=f"bcps{ci}",
                                   tag="bcps")
                nc.tensor.matmul(bc_ps, lhsT=ones128, rhs=pacc,
                                 start=True, stop=True)
                recip_bc = dn_pool.tile([P, CHUNK], BF16, name=f"recbc{ci}",
                                        tag="recbc", bufs=2)
                with nc.allow_low_precision(
                        reason="softmax weights: bf16 rounding ~0.4% << "
                               "2e-2 tol"):
                    nc.vector.reciprocal(recip_bc, bc_ps)
                return recip_bc

            def pass_b_wave(ci, qc, pt_tiles, recip_bc, wave):
                """att^T[do] for do-planes 4*wave..4*wave+3 -> out DMA.
                wave 0 folds the softmax normalization into the P^T tiles."""
                T = 4 * (qc + 1)
                a_ps = [apsum.tile([P, CHUNK], F32, name=f"a{ci}w{wave}d{k}",
                                   tag=f"aw{k}")
                        for k in range(4)]
                for kc in range(T):
                    pt, off = pt_tiles[kc]
                    if wave == 0:
                        nc.vector.tensor_mul(pt[:, off:], pt[:, off:],
                                             recip_bc[:, off:])
                    for k in range(4):
                        do = 4 * wave + k
                        nc.tensor.matmul(
                            a_ps[k][:, off:],
                            lhsT=vc_big[:, kc, do * P:(do + 1) * P],
                            rhs=pt[:, off:],
                            start=(kc == 0), stop=(kc == T - 1))
                # drain the 4 planes into one staging tile, ship with a
                # single strided DMA (sem cascade ~900ns per completion)
                odst = out_d.ap()[:, ci * CHUNK:(ci + 1) * CHUNK]
                odst = odst.rearrange("(g p) q -> p g q", p=P)
                og = out_pool.tile([P, 4, CHUNK], BF16,
                                   name=f"og{ci}w{wave}", tag="og", bufs=4)
                for k in range(4):
                    if k % 2 == 0:
                        nc.vector.tensor_copy(og[:, k, :], a_ps[k])
                    else:
                        nc.scalar.activation(
                            og[:, k, :], a_ps[k],
                            mybir.ActivationFunctionType.Copy)
                nc.sync.dma_start(
                    odst[:, 4 * wave:4 * wave + 4, :], og)

            c0, c1 = chunks[0], chunks[1]
            # S0, S1 back-to-back; bc0 hides behind S1's queue; B0's waves
            # fill the bc1 reciprocal gap and the B1 P^T-fold warmup; B1
            # (the big chunk) runs last but its wave-1 drain overlaps wave
            # transitions
            pt0, pacc0 = pass_a_s(0, c0)
            pt1, pacc1 = pass_a_s(1, c1)
            rbc0 = pass_a_bc(0, pacc0)
            w00 = pass_b_wave(0, c0, pt0, rbc0, 0)
            rbc1 = pass_a_bc(1, pacc1)
            w01 = pass_b_wave(0, c0, pt0, rbc0, 1)
            w10 = pass_b_wave(1, c1, pt1, rbc1, 0)
            w11 = pass_b_wave(1, c1, pt1, rbc1, 1)

# ---------------------------------------------------------------------------
# Host-side dispatch
# ---------------------------------------------------------------------------

_CACHE = {}


def _get_executables():
    if "exes" in _CACHE:
        return _CACHE["exes"]
    import jax
    from jax.sharding import Mesh, PartitionSpec
    from jax.experimental.shard_map import shard_map
    from concourse.bass2jax import (_bass_exec_p, install_neuronx_cc_hook,
                                    partition_id_tensor)

    install_neuronx_cc_hook()
    devices = jax.devices()
    assert len(devices) >= NCORES, f"need {NCORES} devices, have {len(devices)}"

    exes = []
    for half in range(2):
        nc = build(half)
        partition_name = (nc.partition_id_tensor.name
                          if nc.partition_id_tensor else None)
        in_names, out_names, out_avals, zero_shapes = [], [], [], []
        for alloc in nc.m.functions[0].allocations:
            if not isinstance(alloc, mybir.MemoryLocationSet):
                continue
            name = alloc.memorylocations[0].name
            if alloc.kind == "ExternalInput":
                if name != partition_name:
                    in_names.append(name)
            elif alloc.kind == "ExternalOutput":
                out_names.append(name)
                shape = tuple(alloc.tensor_shape)
                dtype = mybir.dt.np(alloc.dtype)
                out_avals.append(jax.core.ShapedArray(shape, dtype))
                zero_shapes.append((shape, dtype))
        n_params = len(in_names)
        all_in_names = list(in_names) + list(out_names)
        if partition_name is not None:
            all_in_names.append(partition_name)
        donate = tuple(range(n_params, n_params + len(out_names)))

        def _body(*args, _nc=nc, _out_avals=tuple(out_avals),
                  _all_in=tuple(all_in_names), _out=tuple(out_names),
                  _pid=partition_name):
            operands = list(args)
            if _pid is not None:
                operands.append(partition_id_tensor())
            return tuple(_bass_exec_p.bind(
                *operands, out_avals=_out_avals, in_names=_all_in,
                out_names=_out, lowering_input_output_aliases=(),
                sim_require_finite=True, sim_require_nnan=True, nc=_nc))

        devs = devices[half * 4:(half + 1) * 4]
        mesh = Mesh(np.asarray(devs), ("core",))
        in_specs = (PartitionSpec("core"),) * (n_params + len(out_names))
        out_specs = (PartitionSpec("core"),) * len(out_names)
        sharded = jax.jit(
            shard_map(_body, mesh=mesh, in_specs=in_specs,
                      out_specs=out_specs, check_rep=False),
            donate_argnums=donate, keep_unused=True)
        exes.append(dict(fn=sharded, in_names=in_names,
                         out_names=out_names, zero_shapes=zero_shapes))
    _CACHE["exes"] = exes
    return exes


def kernel(**inputs):
    x = np.asarray(inputs["x"], dtype=np.float32)      # [B, N, D]
    Wq = np.asarray(inputs["Wq"], dtype=np.float32)
    Wk = np.asarray(inputs["Wk"], dtype=np.float32)
    Wv = np.asarray(inputs["Wv"], dtype=np.float32)
    Wo = np.asarray(inputs["Wo"], dtype=np.float32)

    # algebraic folding: S = x A x^T, out = P (x C)
    A = (Wq @ Wk.T) * np.float32(SCALE)
    C = Wv @ Wo

    import jax
    import time as _time

    exes = _get_executables()
    per_half_in = []
    for half in range(2):
        ex = exes[half]
        per_core = []
        for b in range(B):
            m = {"x": x[b], "a": A, "c": C}
            per_core.append([np.ascontiguousarray(m[nm])
                             for nm in ex["in_names"]])
        per_half_in.append(
            [np.concatenate([per_core[c][i] for c in range(B)], axis=0)
             for i in range(len(ex["in_names"]))])

    outs = None
    for attempt in range(3):
        try:
            outs = []
            for half in range(2):
                ex = exes[half]
                zeros = [np.zeros((B * s[0], *s[1:]), dt)
                         for s, dt in ex["zero_shapes"]]
                outs.append(ex["fn"](*per_half_in[half], *zeros))
            jax.block_until_ready(outs)
            break
        except Exception:
            if attempt == 2:
                raise
            _time.sleep(10)

    out_full = np.empty((B, N, D), dtype=np.float32)
    for half in range(2):
        ex = exes[half]
        arr = np.asarray(outs[half][ex["out_names"].index("out")])
        arr = arr.reshape(B, D, 2 * CHUNK)
        for b in range(B):
            for ci, qc in enumerate(CHUNK_MAP[half]):
                out_full[b, qc * CHUNK:(qc + 1) * CHUNK] = \
                    arr[b, :, ci * CHUNK:(ci + 1) * CHUNK].T
    return out_full



# revision 70
# speedup vs baseline: 1.0946x; 1.0296x over previous
"""Trainium2 Bass kernel for single-head causal attention.

Problem: B=4, N=2048, D=1024, f32.
  Q = x@Wq; K = x@Wk; V = x@Wv (biases are zero in this problem)
  S = Q K^T / sqrt(D), causal-masked, softmax over keys
  out = (softmax(S) @ V) @ Wo

Algebraic folding (host-side): A = Wq Wk^T / sqrt(D), C = Wv Wo.
  S   = x A x^T        -> K projection eliminated (S^T uses resident x^T
                          tiles as the stationary operand)
  out = P (x C) / dn   -> output projection eliminated (denominator scale
                          folds into the attention PSUM->SBUF copy via a
                          matmul-broadcast of the reciprocal row)
This removes 384 of the baseline's 1088 full-rate matmuls per core.

Sharding: 8 cores = (4 batches) x (2 query-halves). Query rows are split in
512-row chunks; core half 0 takes chunks {0,3} of its batch, half 1 takes
{1,2} (both sum to 20 causal key-tiles -> balanced). Causality differs per
half, so we build TWO specialized NEFFs and dispatch them concurrently on
disjoint 4-device meshes. No collectives.

All matmuls use float32r (FP32 storage, FP22 multiply) at free-dim >= 256
where the TensorEngine runs at bf16 rate. Softmax skips the max
subtraction (scores are O(5) here, exp is safe in f32). The softmax
normalization is folded into the P^T tiles (broadcast-reciprocal row x
tile) inside pass B's pipelined region, so the attention-PSUM drain is a
plain copy split across DVE+ACT and never serializes a pass boundary.
PSUM->SBUF drains elsewhere alternate DVE/ACT too. The output is written
transposed ([D, q]) in bf16 (~0.4% rounding, tolerance is 2e-2) and the
host transposes/widens on gather.

The output drain ships 4 do-planes per strided DMA (2 DMAs per chunk
instead of 8): each DMA completion costs ~900ns of semaphore propagation,
which dominated the end-of-kernel cascade. Phase 1+2 interleaves one VC
row-tile between successive x-row transposes so the PE stays busy while
the x stream paces.

The denominator accumulates on DVE directly in f32r storage (f32 bits;
no extra copy) with a single cross-partition matmul per chunk, keeping
the S-chain free of in-order exp waits. VC stream tiles load in pairs with an explicitly
rearrange-pinned DRAM->SBUF mapping (the implicit [2,P,D]->[P,2,D]
mapping scrambles data on hardware).

VC spills ride the scalar queue (slack between the c-odds load and the
late `a` load) so the sync queue streams x rows uninterrupted through
the fine-grained transpose/VC interleave. Constant tiles (identity,
ones, masks) build once outside the For_i repeat loop, so the ~3-5us
Pool-engine constant chain isn't paid per repetition.

Timeline-sim: 189.5 us per half (prior session baseline: 282.7 us; the
wall-clock repeat-loop measurement on these axon-tunneled cores has
+-ms-level noise, so the deterministic sim is the optimization signal).
"""
import sys
import os

sys.path.insert(0, "/opt/trn_rl_repo")

import numpy as np

import concourse.bass as bass
import concourse.mybir as mybir
import concourse.tile as tile
from concourse import bacc
from concourse.masks import make_identity

P = 128
D = 1024
N = 2048
B = 4
NCORES = 8
F32 = mybir.dt.float32
F32R = mybir.dt.float32r
BF16 = mybir.dt.bfloat16
CHUNK = 512           # max query-chunk width (free dim of S^T matmuls)
DSUB = D // P         # 8 feature sub-tiles
NSUB = N // P         # 16 row sub-tiles
SCALE = 1.0 / np.sqrt(D)

# Per-core query chunks as (g0, w) in 128-row tiles. Uneven split
# balances TOTAL per-core PE work: S+att is causally balanced, while
# half 1 never attends keys >= 1536 so it skips 4 VC row-tiles, 4
# transposes and 2MB of x DMA — compensating its wider U (Q-projection).
#   half 0: rows [512:768) + [1536:2048)   (U 6 tiles, VC 16, S+att 18u)
#   half 1: rows [0:512) + [768:1536)      (U 10 tiles, VC 12, S+att 18u)
CHUNK_MAP = [
    [(4, 2), (12, 4)],
    [(0, 4), (6, 4), (10, 2)],
]
NKEYS = [16, 12]      # key tiles (= x rows / VC tiles) touched per half
OUTW = [sum(w for _, w in chs) * P for chs in CHUNK_MAP]   # out cols


def build(half: int, reps: int = 1, parts: str = 'all'):
    """Build the Bass graph for core-half `half` (0 or 1).

    reps > 1 wraps the whole body in a device-side loop - used only for
    wall-clock timing measurements (amortizes the host dispatch overhead).
    """
    chunks = CHUNK_MAP[half]
    nc = bacc.Bacc("TRN2", target_bir_lowering=False, debug=False,
                   enable_asserts=False, num_devices=NCORES // 2)

    x_d = nc.dram_tensor("x", [N, D], F32, kind="ExternalInput")
    a_d = nc.dram_tensor("a", [D, D], F32, kind="ExternalInput")
    # C stays f32: walrus rejects mixed 32/16-bit matmul inputs, and the
    # VC matmul's other operand (x^T) must stay f32r for S precision
    c_d = nc.dram_tensor("c", [D, D], F32, kind="ExternalInput")
    # transposed bf16 output: out^T[d, q] per core; host transposes and
    # widens on gather (bf16 rounding ~0.4% << 2e-2 tolerance)
    out_d = nc.dram_tensor("out", [D, OUTW[half]], BF16,
                           kind="ExternalOutput")

    with tile.TileContext(nc) as tc:
        with tc.tile_pool(name="const", bufs=1) as const:
            # constants build once, OUTSIDE the repeat loop: the ~3-5us
            # Pool-engine constant chain would otherwise re-run every rep.
            # f32 staging tiles live in a scoped pool freed before the body
            with tc.tile_pool(name="cstg", bufs=1) as cstg:
                consts = _build_consts(nc, const, cstg)

            if reps > 1:
                with tc.For_i(0, reps, 1):
                    _build_body(nc, tc, half, chunks, x_d, a_d, c_d, out_d,
                                consts)
            else:
                _build_body(nc, tc, half, chunks, x_d, a_d, c_d, out_d,
                            consts)

    nc.compile()
    return nc


def _build_consts(nc, const, cstg):
    """Constant tiles: PE-transpose identity, all-ones square (one-matmul
    denominator sum+broadcast), and the 4 causal-diagonal mask tiles."""
    # identity for PE transpose (f32r transpose-mode: 1.5 cyc/row);
    # f32 staging tiles share one ring slot (tag) to save SBUF
    ident = cstg.tile([P, P], F32, name="ident", tag="cstg")
    make_identity(nc, ident)
    ident_r = const.tile([P, P], F32R)
    nc.vector.tensor_copy(ident_r, ident)

    # all-ones [P, P]: matmul(lhsT=ones128, rhs=pacc) computes the
    # cross-partition sum AND broadcasts it to all 128 partitions in one
    # instruction (memset can't write f32r; round through DVE)
    ones_f32 = cstg.tile([P, P], F32, name="onesf", tag="cstg")
    nc.gpsimd.memset(ones_f32, 1.0)
    ones128 = const.tile([P, P], F32R)
    nc.vector.tensor_copy(ones128, ones_f32)

    # 4 diagonal mask tiles (bf16, matching the P^T dtype for 2x DVE):
    # M_m[k, q] = 1 if q >= 128*m + k else 0, stored FULL-width from
    # column 0 (narrow chunks have un-trimmable dead zones below the
    # diagonal that must multiply by stored zeros)
    masks = []
    for m in range(4):
        width = P * (m + 1)   # mask is identity-1.0 beyond its diagonal
        mkf = cstg.tile([P, width], F32, name=f"maskf{m}", tag="cstg")
        nc.gpsimd.memset(mkf, 1.0)
        nc.gpsimd.affine_select(
            out=mkf, in_=mkf,
            compare_op=mybir.AluOpType.is_ge,
            fill=0.0,
            base=-P * m,
            channel_multiplier=-1,
            pattern=[[1, width]],
        )
        mk = const.tile([P, width], BF16, name=f"mask{m}")
        nc.vector.tensor_copy(mk, mkf)
        masks.append(mk)
    return dict(ident_r=ident_r, ones128=ones128, masks=masks)


def _build_body(nc, tc, half, chunks, x_d, a_d, c_d, out_d, consts):
    from contextlib import ExitStack

    ident_r = consts["ident_r"]
    ones128 = consts["ones128"]
    masks = consts["masks"]

    ctx = ExitStack()
    with ctx:
        # Outer-scope pools: resident U^T (= A^T x^T, scaled), resident
        # x^T (stays live through phase 3 as the S^T stationary operand),
        # and resident VC = x@C in bf16 (SBUF-resident: no DRAM spill or
        # pass-B reload; bf16 rounding ~0.4% << 2e-2 tol, and the matmul
        # rate is set by the moving operand so a bf16 lhsT costs nothing)
        NK = NKEYS[half]   # key tiles this core touches (x rows, VC tiles)
        qt_pool = ctx.enter_context(tc.tile_pool(name="qtp", bufs=8))
        xt_pool = ctx.enter_context(tc.tile_pool(name="xt", bufs=1))
        vc_pool = ctx.enter_context(tc.tile_pool(name="vcp", bufs=1))
        vc_big = vc_pool.tile([P, NK, D], BF16, name="vcb", tag="vc")

        qt_all = {}    # (ci, d_sub) -> resident U^T tile

        # ============ Phase 1+2: x^T, VC = x@C, U^T = A^T x^T ============
        ph12 = ExitStack()
        with ph12:
            # xld: single-row tiles, depth 5 (per-chunk qt rings freed
            # the SBUF). 1.46us transfers against a 2.34us per-row PE
            # cadence keeps the stream well ahead.
            xld = ph12.enter_context(tc.tile_pool(name="xld", bufs=5))
            cpool = ph12.enter_context(tc.tile_pool(name="cp", bufs=1))
            apool = ph12.enter_context(tc.tile_pool(name="ap", bufs=1))
            tpsum = ph12.enter_context(
                tc.tile_pool(name="tpsum", bufs=2, space="PSUM"))
            ppsum = ph12.enter_context(
                tc.tile_pool(name="ppsum", bufs=5, space="PSUM"))

            xt_big = xt_pool.tile([P, DSUB, NK * P], F32R, name="xtb",
                                  tag="xt")
            xt = [xt_big[:, i, :] for i in range(DSUB)]

            xrows = {}   # row index -> staged [P, D] tile

            def load_xrow(j, split_first=False):
                xsrc = x_d.ap().bitcast(F32R)
                xr = xld.tile([P, D], F32R, name="xr", tag="xrow")
                if split_first:
                    # fine-grained row 0: the first transpose only needs
                    # columns 0:128, so land that piece ASAP
                    for c0, c1 in ((0, P), (P, CHUNK), (CHUNK, D)):
                        nc.sync.dma_start(xr[:, c0:c1],
                                          xsrc[0:P, c0:c1])
                else:
                    nc.sync.dma_start(xr, xsrc[j * P:(j + 1) * P, :])
                xrows[j] = xr

            def transpose_batch(jlist):
                # 4 [128,128] transposes share one full PSUM bank and drain
                # with a single wide DVE copy (3D AP scatters the 4 blocks
                # into the i-planes of xt_big)
                for j in jlist:
                    xrow = xrows[j]
                    for i4 in range(0, DSUB, 4):
                        tp = tpsum.tile([P, 4 * P], F32R, name="tp", tag="tp")
                        for k in range(4):
                            nc.tensor.transpose(
                                tp[:, k * P:(k + 1) * P],
                                xrow[:, (i4 + k) * P:(i4 + k + 1) * P], ident_r)
                        if i4 == 0:
                            nc.vector.tensor_copy(
                                xt_big[:, i4:i4 + 4, j * P:(j + 1) * P],
                                tp.rearrange("p (a b) -> p a b", a=4))
                        else:
                            nc.scalar.activation(
                                xt_big[:, i4:i4 + 4, j * P:(j + 1) * P],
                                tp.rearrange("p (a b) -> p a b", a=4),
                                mybir.ActivationFunctionType.Copy)

            # C rides the scalar queue in column-eighth descriptors into
            # one [P, 8, D] resident tile. Only the LEFT half (4 eighths)
            # loads up front: the VC left-half passes below need just
            # cw[:, 0:512], so PE-feeding work exists ~6us in. The right
            # half is paced into the scalar queue mid-loop, and A follows
            # it — neither steals bandwidth from the x stream while the x
            # stream is the critical path.
            cw_big = cpool.tile([P, DSUB, D], F32R, name="cw", tag="w")
            csrc = c_d.ap().bitcast(F32R)
            CQ = D // 8

            def load_c_eighth(h, eng=None):
                (eng or nc.scalar).dma_start(
                    cw_big[:, :, h * CQ:(h + 1) * CQ],
                    csrc[:, h * CQ:(h + 1) * CQ].rearrange(
                        "(s p) d -> p s d", p=P))

            def vc_half(ns, dh):
                # one column-half of one VC row-tile: 8 matmuls, 1.7us PE
                ps = ppsum.tile([P, CHUNK], F32, name="vps", tag="pp")
                for di in range(DSUB):
                    nc.tensor.matmul(
                        ps,
                        lhsT=xt[di][:, ns * P:(ns + 1) * P],
                        rhs=cw[di][:, dh * CHUNK:(dh + 1) * CHUNK],
                        start=(di == 0), stop=(di == DSUB - 1))
                # drain straight into the resident bf16 VC tile,
                # alternating DVE/ACT
                if dh == 0:
                    nc.vector.tensor_copy(
                        vc_big[:, ns, dh * CHUNK:(dh + 1) * CHUNK], ps)
                else:
                    nc.scalar.activation(
                        vc_big[:, ns, dh * CHUNK:(dh + 1) * CHUNK],
                        ps, mybir.ActivationFunctionType.Copy)

            # row 0 first (unlocks T0), then C-left on BOTH queues at
            # full bandwidth — PE has no other work until C-left lands,
            # so starving the x stream for 5.8us here costs nothing
            load_xrow(0, split_first=True)
            for h in range(4):
                load_c_eighth(h)
            cw = [cw_big[:, s, :] for s in range(DSUB)]

            load_xrow(1)
            transpose_batch(range(0, 2))
            vc_half(0, 0)

            # Pass 1: transposes interleaved with VC LEFT halves (only
            # need C's left half, in ~6us); lag 1 — row j's VC-half runs
            # after transpose j+1, giving the T drains (DVE/ACT) one
            # transpose of slack without waiting on far-later rows. C
            # right-half eighths are paced into the scalar queue mid-loop
            # to soak leftover HBM bandwidth.
            c_pace = (7, 10, 13) if NK == 16 else (6, 8, 10)
            for j in range(2, NK):
                load_xrow(j)
                transpose_batch(range(j, j + 1))
                vc_half(j - 1, 0)
                if j in c_pace:
                    load_c_eighth(4 + c_pace.index(j))
            vc_half(NK - 1, 0)
            load_c_eighth(7)

            # Pass 2: VC right halves; A follows C on the scalar queue a
            # few groups in — its ~11.6us transfer runs under the VC
            # right-half passes and lands well before U^T
            aw_big = apool.tile([P, DSUB, D], F32R, name="aw", tag="w")
            asrc = a_d.ap().bitcast(F32R).rearrange("(s p) d -> p s d", p=P)
            for ns in range(NK):
                vc_half(ns, 1)
                if ns in (2, 4, 6, 8):
                    s2 = ns - 2
                    nc.scalar.dma_start(aw_big[:, s2:s2 + 2, :],
                                        asrc[:, s2:s2 + 2, :])
            aw = [aw_big[:, s, :] for s in range(DSUB)]

            # --- U^T = A^T x^T at my query chunks (scale folded into A) ---
            for ci, (g0t, w) in enumerate(chunks):
                g0 = g0t * P
                for do in range(DSUB):
                    ps = ppsum.tile([P, w * P], F32, name="qtps", tag="pp")
                    for di in range(DSUB):
                        nc.tensor.matmul(
                            ps,
                            lhsT=aw[di][:, do * P:(do + 1) * P],
                            rhs=xt[di][:, g0:g0 + w * P],
                            start=(di == 0), stop=(di == DSUB - 1))
                    qts = qt_pool.tile([P, w * P], F32R,
                                       name=f"qt{ci}_{do}", tag=f"qt{ci}",
                                       bufs=8)
                    if do % 2 == 0:
                        nc.vector.tensor_copy(qts, ps)
                    else:
                        nc.scalar.activation(
                            qts, ps, mybir.ActivationFunctionType.Copy)
                    qt_all[(ci, do)] = qts

        # ================= Phase 3: attention per q-chunk =================
        # Reordered to keep the in-order PE queue busy across the serial
        # PE->DVE->PE denominator chains: both chunks' S-passes run
        # back-to-back, then pass B interleaves the small chunk's waves
        # into the chain gaps. Pass B is split into two 4-bank PSUM waves
        # (do-planes 0..3 / 4..7) so spsum+dpsum+apsum fit in 8 banks and
        # a wave's drain overlaps the next wave's matmuls.
        ph3 = ExitStack()
        with ph3:
            pt_pool = ph3.enter_context(tc.tile_pool(name="ptp", bufs=26))
            out_pool = ph3.enter_context(tc.tile_pool(name="outp", bufs=4))
            dn_pool = ph3.enter_context(tc.tile_pool(name="dnp", bufs=6))
            # bufs is the per-tag ring depth: spsum 1 tag x 2 = 2 banks,
            # dpsum 2 tags x 1 = 2 banks, apsum 4 tags x 1 = 4 banks
            spsum = ph3.enter_context(
                tc.tile_pool(name="spsum", bufs=2, space="PSUM"))
            dpsum = ph3.enter_context(
                tc.tile_pool(name="dpsum", bufs=1, space="PSUM"))
            apsum = ph3.enter_context(
                tc.tile_pool(name="apsum", bufs=1, space="PSUM"))

            def pass_a_s(ci):
                """S^T -> exp -> P^T tiles; DVE denominator acc."""
                g0t, w = chunks[ci]
                W = w * P
                T = g0t + w            # causal key tiles for this chunk
                qt = [qt_all[(ci, s)] for s in range(DSUB)]
                pacc = dn_pool.tile([P, W], F32R, name=f"pacc{ci}",
                                    tag="pacc", bufs=2)

                # diagonal tiles FIRST: their serial exp->mask->pacc tail
                # then runs while the off-diagonal S matmuls stream, so the
                # bc matmul at the end only waits on a plain add. P^T is
                # bf16 (walrus requires pass B's operands to match vc_big's
                # 16-bit class; also 2x DVE throughput on the softmax chain)
                kc_order = list(range(g0t, T)) + list(range(g0t))
                pt_by_kc = {}
                for ki, kc in enumerate(kc_order):
                    m = kc - g0t
                    # block-causal: diagonal tile m only touches query
                    # columns >= 128*m (cap so the live width stays >= 256,
                    # where fp32r runs at full rate)
                    off = max(0, min(P * m, W - 2 * P)) if m > 0 else 0
                    ps = spsum.tile([P, W], F32, name="sps", tag="sp")
                    for s in range(DSUB):
                        nc.tensor.matmul(
                            ps[:, off:],
                            lhsT=xt[s][:, kc * P:(kc + 1) * P],
                            rhs=qt[s][:, off:],
                            start=(s == 0), stop=(s == DSUB - 1))
                    pt = pt_pool.tile([P, W], BF16, name="pt",
                                      tag="pt")
                    nc.scalar.activation(
                        pt[:, off:], ps[:, off:],
                        mybir.ActivationFunctionType.Exp)
                    if 0 <= m < w:
                        # mask m zeroes below the diagonal in its 128-col
                        # window AND any un-trimmed dead zone left of it
                        hi = min(P * (m + 1), W)
                        if hi > off:
                            nc.vector.tensor_mul(pt[:, off:hi],
                                                 pt[:, off:hi],
                                                 masks[m][:, off:hi])
                    pt_by_kc[kc] = (pt, off)
                    # denominator partial sums on DVE so the S-chain
                    # never stalls in-order behind exp/mask; diagonal
                    # tiles add only their live columns (f32r accumulator
                    # keeps the 2048-term sum accurate)
                    if ki == 0:
                        nc.vector.tensor_copy(pacc, pt)
                    else:
                        nc.vector.tensor_add(pacc[:, off:],
                                             pacc[:, off:],
                                             pt[:, off:])
                pt_tiles = [pt_by_kc[kc] for kc in range(T)]
                return pt_tiles, pacc

            def pass_a_bc(ci, pacc):
                """One matmul sums pacc across partitions AND broadcasts the
                denominator to all 128; reciprocal straight to bf16."""
                W = chunks[ci][1] * P
                bc_ps = dpsum.tile([P, W], F32, name=f"bcps{ci}",
                                   tag="bcps")
                nc.tensor.matmul(bc_ps, lhsT=ones128, rhs=pacc,
                                 start=True, stop=True)
                recip_bc = dn_pool.tile([P, W], BF16, name=f"recbc{ci}",
                                        tag="recbc", bufs=2)
                with nc.allow_low_precision(
                        reason="softmax weights: bf16 rounding ~0.4% << "
                               "2e-2 tol"):
                    nc.vector.reciprocal(recip_bc, bc_ps)
                return recip_bc

            # out column offset of each chunk in out_d
            col_ofs = []
            acc = 0
            for g0t, w in chunks:
                col_ofs.append(acc)
                acc += w * P

            def pass_b_wave(ci, pt_tiles, recip_bc, wave):
                """att^T[do] for do-planes 4*wave..4*wave+3 -> out DMA.
                wave 0 folds the softmax normalization into the P^T tiles."""
                g0t, w = chunks[ci]
                W = w * P
                T = g0t + w
                a_ps = [apsum.tile([P, W], F32, name=f"a{ci}w{wave}d{k}",
                                   tag=f"aw{k}")
                        for k in range(4)]
                for kc in range(T):
                    pt, off = pt_tiles[kc]
                    if wave == 0:
                        nc.vector.tensor_mul(pt[:, off:], pt[:, off:],
                                             recip_bc[:, off:])
                    for k in range(4):
                        do = 4 * wave + k
                        nc.tensor.matmul(
                            a_ps[k][:, off:],
                            lhsT=vc_big[:, kc, do * P:(do + 1) * P],
                            rhs=pt[:, off:],
                            start=(kc == 0), stop=(kc == T - 1))
                # drain the 4 planes into one staging tile, ship with a
                # single strided DMA (sem cascade ~900ns per completion)
                odst = out_d.ap()[:, col_ofs[ci]:col_ofs[ci] + W]
                odst = odst.rearrange("(g p) q -> p g q", p=P)
                og = out_pool.tile([P, 4, W], BF16,
                                   name=f"og{ci}w{wave}", tag="og", bufs=4)
                for k in range(4):
                    if k % 2 == 0:
                        nc.vector.tensor_copy(og[:, k, :], a_ps[k])
                    else:
                        nc.scalar.activation(
                            og[:, k, :], a_ps[k],
                            mybir.ActivationFunctionType.Copy)
                nc.sync.dma_start(
                    odst[:, 4 * wave:4 * wave + 4, :], og)

            # All S-passes back-to-back; each chunk's bc hides behind the
            # next S-pass or an earlier chunk's B-wave; B-waves of earlier
            # (smaller) chunks fill the reciprocal/fold warmup gaps of
            # later ones
            n = len(chunks)
            pts, paccs = [], []
            for ci in range(n):
                pt_i, pacc_i = pass_a_s(ci)
                pts.append(pt_i)
                paccs.append(pacc_i)
            # wave order: chunk 0's wave 0 first (covers chunk 1's
            # reciprocal chain), chunk 0's wave 1 LAST — chunk 0 is the
            # narrowest/shallowest chunk, so the end-of-kernel drain tail
            # is as short as possible
            rbcs = [None] * n
            rbcs[0] = pass_a_bc(0, paccs[0])
            pass_b_wave(0, pts[0], rbcs[0], 0)
            for ci in range(1, n):
                rbcs[ci] = pass_a_bc(ci, paccs[ci])
                pass_b_wave(ci, pts[ci], rbcs[ci], 0)
                pass_b_wave(ci, pts[ci], rbcs[ci], 1)
            pass_b_wave(0, pts[0], rbcs[0], 1)

# ---------------------------------------------------------------------------
# Host-side dispatch
# ---------------------------------------------------------------------------

_CACHE = {}


def _get_executables():
    if "exes" in _CACHE:
        return _CACHE["exes"]
    import jax
    from jax.sharding import Mesh, PartitionSpec
    from jax.experimental.shard_map import shard_map
    from concourse.bass2jax import (_bass_exec_p, install_neuronx_cc_hook,
                                    partition_id_tensor)

    install_neuronx_cc_hook()
    devices = jax.devices()
    assert len(devices) >= NCORES, f"need {NCORES} devices, have {len(devices)}"

    exes = []
    for half in range(2):
        nc = build(half)
        partition_name = (nc.partition_id_tensor.name
                          if nc.partition_id_tensor else None)
        in_names, out_names, out_avals, zero_shapes = [], [], [], []
        for alloc in nc.m.functions[0].allocations:
            if not isinstance(alloc, mybir.MemoryLocationSet):
                continue
            name = alloc.memorylocations[0].name
            if alloc.kind == "ExternalInput":
                if name != partition_name:
                    in_names.append(name)
            elif alloc.kind == "ExternalOutput":
                out_names.append(name)
                shape = tuple(alloc.tensor_shape)
                dtype = mybir.dt.np(alloc.dtype)
                out_avals.append(jax.core.ShapedArray(shape, dtype))
                zero_shapes.append((shape, dtype))
        n_params = len(in_names)
        all_in_names = list(in_names) + list(out_names)
        if partition_name is not None:
            all_in_names.append(partition_name)
        donate = tuple(range(n_params, n_params + len(out_names)))

        def _body(*args, _nc=nc, _out_avals=tuple(out_avals),
                  _all_in=tuple(all_in_names), _out=tuple(out_names),
                  _pid=partition_name):
            operands = list(args)
            if _pid is not None:
                operands.append(partition_id_tensor())
            return tuple(_bass_exec_p.bind(
                *operands, out_avals=_out_avals, in_names=_all_in,
                out_names=_out, lowering_input_output_aliases=(),
                sim_require_finite=True, sim_require_nnan=True, nc=_nc))

        devs = devices[half * 4:(half + 1) * 4]
        mesh = Mesh(np.asarray(devs), ("core",))
        in_specs = (PartitionSpec("core"),) * (n_params + len(out_names))
        out_specs = (PartitionSpec("core"),) * len(out_names)
        sharded = jax.jit(
            shard_map(_body, mesh=mesh, in_specs=in_specs,
                      out_specs=out_specs, check_rep=False),
            donate_argnums=donate, keep_unused=True)
        exes.append(dict(fn=sharded, in_names=in_names,
                         out_names=out_names, zero_shapes=zero_shapes))
    _CACHE["exes"] = exes
    return exes


def kernel(**inputs):
    x = np.asarray(inputs["x"], dtype=np.float32)      # [B, N, D]
    Wq = np.asarray(inputs["Wq"], dtype=np.float32)
    Wk = np.asarray(inputs["Wk"], dtype=np.float32)
    Wv = np.asarray(inputs["Wv"], dtype=np.float32)
    Wo = np.asarray(inputs["Wo"], dtype=np.float32)

    # algebraic folding: S = x A x^T, out = P (x C)
    A = (Wq @ Wk.T) * np.float32(SCALE)
    C = Wv @ Wo

    import jax
    import time as _time

    exes = _get_executables()
    per_half_in = []
    for half in range(2):
        ex = exes[half]
        per_core = []
        for b in range(B):
            m = {"x": x[b], "a": A, "c": C}
            per_core.append([np.ascontiguousarray(m[nm])
                             for nm in ex["in_names"]])
        per_half_in.append(
            [np.concatenate([per_core[c][i] for c in range(B)], axis=0)
             for i in range(len(ex["in_names"]))])

    outs = None
    for attempt in range(3):
        try:
            outs = []
            for half in range(2):
                ex = exes[half]
                zeros = [np.zeros((B * s[0], *s[1:]), dt)
                         for s, dt in ex["zero_shapes"]]
                outs.append(ex["fn"](*per_half_in[half], *zeros))
            jax.block_until_ready(outs)
            break
        except Exception:
            if attempt == 2:
                raise
            _time.sleep(10)

    out_full = np.empty((B, N, D), dtype=np.float32)
    for half in range(2):
        ex = exes[half]
        arr = np.asarray(outs[half][ex["out_names"].index("out")])
        arr = arr.reshape(B, D, OUTW[half])
        for b in range(B):
            ofs = 0
            for g0t, w in CHUNK_MAP[half]:
                out_full[b, g0t * P:(g0t + w) * P] = \
                    arr[b, :, ofs:ofs + w * P].T
                ofs += w * P
    return out_full

